# revision 14
# baseline (speedup 1.0000x reference)
"""Trainium2 Bass kernel for nn_CorrBlock (sparse_attention).

Reference semantics (including its scrambled reshape): with
f = einsum('btchw,oc->bothw', ...) reshaped as (b*t, c2, h*w), slab
n = B*8+nb contracts over channels O in [20*nb, 20*nb+20) x all 8 frame
pairs tau.  Per slab the computation is: corr = f0^T @ f1 / 160 on a
1024x1024 grid, a 2-level pyramid (2x2 avg pool of the j-grid), and a
7x7 window lookup around each query pixel (bilinear, zero padding).

Kernel strategy (8 cores, data parallel):
  core cid -> batch B = cid//2, channel block O in [80*(cid%2), +80),
  i.e. 4 slabs per core; all 9 frames of batch B are inputs.

  - 1x1 convs as PE matmuls (fp32r = full-rate fp32), psum -> SBUF
    staging copy, then SBUF->SBUF DMAs reshuffle rows into slab-major
    tiles (row c' = tau*20 + oo).
  - Level-0 corr: windowed matmul per 128-query chunk (only the +-99
    diagonal band is ever needed), then a "skew" DRAM round trip:
    write rows at pitch 326, read back at pitch 327 -- this turns the
    per-query diagonal window into a plain strided access.  One masked
    DVE op extracts all 49 window taps.
  - Level-1: 2x2 pooling and the y-direction bilinear are folded into
    the matmul rhs (zero-padded 24x16 grid); queries are processed in
    parity classes so the same skew trick applies; x-direction bilinear
    is 1-3 DVE ops with host-precomputed masked weights.
"""
import sys

if "/opt/trn_rl_repo" not in sys.path:
    sys.path.insert(0, "/opt/trn_rl_repo")

import numpy as np

import concourse.mybir as mybir
from concourse import bacc
from concourse.tile import TileContext
from concourse.bass_utils import run_bass_kernel_spmd

F32 = mybir.dt.float32
F32R = mybir.dt.float32r

B, C, T, H, W = 4, 320, 9, 32, 32
TP = T - 1
C2 = 160
HW = H * W
SLABS = 4            # slabs (output t positions) per core
K49 = 49
KC = [(0, 128), (128, 128), (256, 64)]   # contraction chunks over C=320
BW0 = 199            # level-0 band width (offsets -99..99)
WR0 = BW0 + 127      # 326 skew write width
EXTW = 24 * 16       # 384: zero-padded level-1 grid (y2e in -4..19)
BW1 = 104            # level-1 band width
WR1 = BW1 + 127      # 231
SCL0 = 1.0 / C2
SCL1 = 0.25 / C2

USE_F32R = True
MMDT = F32R if USE_F32R else F32

_CACHE = {}
LAST_RESULTS = None


def _vec(pairs):
    return mybir.VecI64Pair([tuple(p) for p in pairs])


def _cap(ap, dims, extra_off=0):
    """Custom AP: keep tensor/offset of `ap`, replace dims."""
    a = ap.copy()
    a.ap = _vec(dims)
    a.offset = ap.offset + extra_off
    return a


def _win(ap, free_dims, extra_off=0):
    """Custom AP keeping the partition dim of `ap`, new free dims."""
    p = ap.ap[0]
    return _cap(ap, [tuple(p)] + [tuple(d) for d in free_dims], extra_off)


def _consts():
    d = np.arange(-3, 4)
    dxk = np.repeat(d, 7)   # x offset indexed by k//7
    dyk = np.tile(d, 7)     # y offset indexed by k%7
    yv = np.arange(H)[:, None]
    xv = np.arange(W)[:, None]
    vx = (xv + dxk[None, :] >= 0) & (xv + dxk[None, :] < W)
    vy = (yv + dyk[None, :] >= 0) & (yv + dyk[None, :] < H)
    mask0 = (vy[:, None, :] & vx[None, :, :]).astype(np.float32).reshape(HW, K49)
    xx = np.arange(16)[:, None]
    mx0 = ((xx + dxk[None, :] >= 0) & (xx + dxk[None, :] < 16)).astype(np.float32)
    mx1 = ((xx + dxk[None, :] + 1 >= 0) & (xx + dxk[None, :] + 1 < 16)).astype(np.float32)
    mx0 = np.tile(mx0, (8, 1))
    mx1 = np.tile(mx1, (8, 1))
    wmx = np.stack([mx0, 0.5 * mx0, 0.25 * mx0, 0.5 * mx1, 0.25 * mx1])
    return np.ascontiguousarray(mask0), np.ascontiguousarray(wmx)


def _body(nc, tc, xin, w0t, w1t, mask0d, wmxd, out0d, out1d):
    from contextlib import ExitStack

    mmdt = F32R if USE_F32R else F32

    def mm(out_ap, lhs_ap, rhs_ap, start, stop):
        nc.tensor.matmul(out_ap, lhs_ap.bitcast(mmdt), rhs_ap.bitcast(mmdt),
                         start=start, stop=stop)

    with ExitStack() as ex:
        cpool = ex.enter_context(tc.tile_pool(name="consts", bufs=1))
        fpool = ex.enter_context(tc.tile_pool(name="fslab", bufs=1))
        xpool = ex.enter_context(tc.tile_pool(name="xin", bufs=2))
        spool = ex.enter_context(tc.tile_pool(name="stg", bufs=3))
        wpool = ex.enter_context(tc.tile_pool(name="work", bufs=2))
        wbig = ex.enter_context(tc.tile_pool(name="wbig", bufs=1))
        opool = ex.enter_context(tc.tile_pool(name="outp", bufs=2))
        pfpool = ex.enter_context(tc.tile_pool(name="psf", bufs=2, space="PSUM"))
        pcpool = ex.enter_context(tc.tile_pool(name="psc", bufs=2, space="PSUM"))
        dpool = ex.enter_context(tc.tile_pool(name="dstage", bufs=3, space="DRAM"))

        # ---------------- constants ----------------
        wt = {}
        for wi, wd in ((0, w0t), (1, w1t)):
            for ci, (k0, kn) in enumerate(KC):
                t = cpool.tile([kn, 80], MMDT, tag=f"w{wi}_{ci}")
                nc.sync.dma_start(out=t[:, :], in_=wd[k0:k0 + kn, :].bitcast(MMDT))
                wt[(wi, ci)] = t
        m0t = cpool.tile([128, 8 * K49], F32, tag="mask0")
        nc.sync.dma_start(
            out=m0t[:, :],
            in_=_cap(mask0d[:, :], [(K49, 128), (128 * K49, 8), (1, K49)]))
        wmt = cpool.tile([128, 5 * K49], F32, tag="wmx")
        nc.sync.dma_start(
            out=wmt[:, :],
            in_=_cap(wmxd[:, :, :], [(K49, 128), (128 * K49, 5), (1, K49)]))

        # slab-major feature tiles; slab s occupies cols [s*1024, (s+1)*1024)
        f0a = fpool.tile([128, SLABS * HW], MMDT, tag="f0a")
        f1a = fpool.tile([128, SLABS * HW], MMDT, tag="f1a")
        # class-permuted f0 (for level 1): col cl*256 + (y//2)*16 + (x//2)
        f0pa = fpool.tile([128, SLABS * HW], MMDT, tag="f0pa")
        f0b = fpool.tile([32, SLABS * HW], MMDT, tag="f0b")
        f1b = fpool.tile([32, SLABS * HW], MMDT, tag="f1b")
        f0pb = fpool.tile([32, SLABS * HW], MMDT, tag="f0pb")

        # ---------------- convs over the 9 frames ----------------
        # DRAM staging per weight, slab-major: offset = s*(TP*20*HW) + rtau*20*HW + oo*HW + i
        SLABSZ = TP * 20 * HW      # per-slab block in the staging buffer
        stgd = {}
        for wi in (0, 1):
            stgd_t = dpool.tile([1, SLABS * SLABSZ], MMDT, tag=f"stgd{wi}")
            stgd[wi] = stgd_t
        for tau in range(T):
            xts = []
            for ci, (k0, kn) in enumerate(KC):
                t = xpool.tile([kn, HW], MMDT, tag=f"x{ci}")
                nc.sync.dma_start(out=t[:, :], in_=xin[k0:k0 + kn, tau, :].bitcast(MMDT))
                xts.append(t)
            for wi in (0, 1):
                if wi == 0 and tau > TP - 1:
                    continue
                if wi == 1 and tau < 1:
                    continue
                ps = pfpool.tile([80, 1024], F32, tag="psf")
                for nb in (0, 1):
                    for ci in range(3):
                        mm(ps[:, nb * 512:(nb + 1) * 512],
                           wt[(wi, ci)][:, :],
                           xts[ci][:, nb * 512:(nb + 1) * 512],
                           start=(ci == 0), stop=(ci == 2))
                stg = spool.tile([80, 1024], MMDT, tag="stg")
                nc.vector.tensor_copy(stg[:, :], ps[:, :])
                rtau = tau if wi == 0 else tau - 1
                nc.sync.dma_start(
                    out=_cap(stgd[wi][:, :], [(SLABSZ, SLABS), (HW, 20), (1, HW)],
                             rtau * 20 * HW),
                    in_=stg[:, :])
        # batched reads: slab-row-major staging makes rows contiguous per slab
        for wi, (fa, fb) in ((0, (f0a, f0b)), (1, (f1a, f1b))):
            nc.sync.dma_start(
                out=fa[:, :],
                in_=_cap(stgd[wi][:, :], [(HW, 128), (SLABSZ, SLABS), (1, HW)]))
            nc.sync.dma_start(
                out=fb[:, :],
                in_=_cap(stgd[wi][:, :], [(HW, 32), (SLABSZ, SLABS), (1, HW)],
                         128 * HW))

        # ---------------- per-slab processing ----------------
        for s in range(SLABS):
            sc = s * HW

            # permuted f0 for this slab: iterate (py, y2, x2) per px
            for src, dst in ((f0a[:, :], f0pa[:, :]), (f0b[:, :], f0pb[:, :])):
                for px in range(2):
                    nc.vector.tensor_copy(
                        _win(dst, [(512, 2), (16, 16), (1, 16)], sc + px * 256),
                        _win(src, [(32, 2), (64, 16), (2, 16)], sc + px))

            # level-1 rhs: pooled f1 on the zero-padded 24x16 grid + y-stencil
            exts, hhs = [], []
            for src, kn, tg in ((f1a[:, :], 128, "a"), (f1b[:, :], 32, "b")):
                ext = wpool.tile([kn, EXTW], MMDT, tag=f"ext{tg}")
                hh = wpool.tile([kn, EXTW], MMDT, tag=f"hh{tg}")
                nc.gpsimd.memset(ext[:, :].bitcast(F32), 0.0)
                dst = ext[:, 64:320]
                t0 = _win(src, [(64, 16), (2, 16)], sc + 0)
                t1 = _win(src, [(64, 16), (2, 16)], sc + 1)
                t2 = _win(src, [(64, 16), (2, 16)], sc + 32)
                t3 = _win(src, [(64, 16), (2, 16)], sc + 33)
                nc.vector.tensor_add(dst, t0, t1)
                nc.vector.tensor_add(dst, dst, t2)
                nc.vector.tensor_add(dst, dst, t3)
                nc.gpsimd.memset(hh[:, 368:384].bitcast(F32), 0.0)
                nc.vector.tensor_add(hh[:, 0:368], ext[:, 0:368], ext[:, 16:384])
                exts.append(ext)
                hhs.append(hh)

            o0 = opool.tile([128, 8 * K49], F32, tag="o0")
            o1 = opool.tile([128, 8 * K49], F32, tag="o1")

            # ---- level 0: banded corr + skew + mask ----
            wide0 = wbig.tile([128, 8 * WR0], F32, tag="wide0")
            for ic in range(8):
                wlo = max(0, ic * 128 - 99)
                whi = min(HW, ic * 128 + 227)
                n = whi - wlo
                lo = wlo - (ic * 128 - 99)
                # fp32r matmuls need an even moving-dim; pad inside the matrix
                src_off = 0
                mlo, mhi = wlo, whi
                if n % 2:
                    if mlo > 0:
                        mlo -= 1
                        src_off = 1
                    else:
                        mhi += 1
                nmm = mhi - mlo
                ps = pcpool.tile([128, WR0], F32, tag="ps0")
                mm(ps[:, 0:nmm], f0a[:, sc + ic * 128:sc + (ic + 1) * 128],
                   f1a[:, sc + mlo:sc + mhi], True, False)
                mm(ps[:, 0:nmm], f0b[:, sc + ic * 128:sc + (ic + 1) * 128],
                   f1b[:, sc + mlo:sc + mhi], False, True)
                wc = ic * WR0
                if lo > 0 or n < WR0:
                    nc.gpsimd.memset(wide0[:, wc:wc + WR0], 0.0)
                nc.scalar.mul(wide0[:, wc + lo:wc + lo + n],
                              ps[:, src_off:src_off + n], SCL0)
            sk = dpool.tile([1, 8 * 128 * WR0], F32, tag="sk0")
            nc.sync.dma_start(
                out=_cap(sk[:, :], [(WR0, 128), (128 * WR0, 8), (1, WR0)]),
                in_=wide0[:, :])
            band0 = wbig.tile([128, 8 * BW0], F32, tag="band0")
            nc.sync.dma_start(
                out=band0[:, :],
                in_=_cap(sk[:, :], [(WR0 + 1, 128), (128 * WR0, 8), (1, BW0)]))
            for ic in range(8):
                nc.vector.tensor_mul(o0[:, ic * K49:(ic + 1) * K49],
                                     _win(band0[:, :], [(1, 7), (32, 7)], ic * BW0),
                                     m0t[:, ic * K49:(ic + 1) * K49])

            # ---- level 1: class-permuted pooled corr + skew + x-combine ----
            wide1 = wbig.tile([128, 8 * WR1], F32, tag="wide1")
            for cl in range(4):
                py = cl // 2
                rhs = exts if py == 0 else hhs
                for half in range(2):
                    ps = pcpool.tile([128, 256], F32, tag="ps1")
                    co = sc + cl * 256 + 128 * half
                    wlo = 128 * half
                    mm(ps[:, :], f0pa[:, co:co + 128], rhs[0][:, wlo:wlo + 256], True, False)
                    mm(ps[:, :], f0pb[:, co:co + 128],
                       rhs[1][:, wlo:wlo + 256], False, True)
                    g = cl * 2 + half
                    nc.scalar.mul(wide1[:, g * WR1:(g + 1) * WR1],
                                  ps[:, 13:13 + WR1], SCL1)
            sk1 = dpool.tile([1, 8 * 128 * WR1], F32, tag="sk1")
            nc.sync.dma_start(
                out=_cap(sk1[:, :], [(WR1, 128), (128 * WR1, 8), (1, WR1)]),
                in_=wide1[:, :])
            band1 = wbig.tile([128, 8 * BW1], F32, tag="band1")
            nc.sync.dma_start(
                out=band1[:, :],
                in_=_cap(sk1[:, :], [(WR1 + 1, 128), (128 * WR1, 8), (1, BW1)]))
            for cl in range(4):
                py, px = cl // 2, cl % 2
                for half in range(2):
                    g = cl * 2 + half
                    oc = g * K49
                    s0ap = _win(band1[:, :], [(1, 7), (16, 7)], g * BW1)
                    s1ap = _win(band1[:, :], [(1, 7), (16, 7)], g * BW1 + 1)
                    if px == 0:
                        w0i = 0 if py == 0 else 1
                        nc.vector.tensor_mul(o1[:, oc:oc + K49], s0ap,
                                             wmt[:, w0i * K49:(w0i + 1) * K49])
                    else:
                        w0i = 1 if py == 0 else 2
                        w1i = 3 if py == 0 else 4
                        tmp = opool.tile([128, K49], F32, tag="tmp49")
                        nc.vector.tensor_mul(tmp[:, :], s0ap,
                                             wmt[:, w0i * K49:(w0i + 1) * K49])
                        nc.vector.tensor_mul(o1[:, oc:oc + K49], s1ap,
                                             wmt[:, w1i * K49:(w1i + 1) * K49])
                        nc.vector.tensor_add(o1[:, oc:oc + K49],
                                             o1[:, oc:oc + K49], tmp[:, :])

            nc.sync.dma_start(
                out=_cap(out0d[s], [(K49, 128), (128 * K49, 8), (1, K49)]),
                in_=o0[:, :])
            nc.sync.dma_start(
                out=_cap(out1d[s], [(K49, 128), (128 * K49, 8), (1, K49)]),
                in_=o1[:, :])


def _build(reps=1):
    key = f"nc{reps}"
    if key in _CACHE:
        return _CACHE[key]
    nc = bacc.Bacc("TRN2", target_bir_lowering=False, debug=False)
    xin = nc.dram_tensor("xin", [C, T, HW], F32, kind="ExternalInput")
    w0t = nc.dram_tensor("w0t", [C, 80], F32, kind="ExternalInput")
    w1t = nc.dram_tensor("w1t", [C, 80], F32, kind="ExternalInput")
    mask0d = nc.dram_tensor("mask0", [HW, K49], F32, kind="ExternalInput")
    wmxd = nc.dram_tensor("wmx", [5, 128, K49], F32, kind="ExternalInput")
    out0d = nc.dram_tensor("out0", [SLABS, 8, 128, K49], F32, kind="ExternalOutput")
    out1d = nc.dram_tensor("out1", [SLABS, 4, 2, 128, K49], F32, kind="ExternalOutput")
    with TileContext(nc) as tc:
        for _ in range(reps):
            _body(nc, tc, xin, w0t, w1t, mask0d, wmxd, out0d, out1d)
    nc.compile()
    _CACHE[key] = nc
    return nc


def kernel(x, w_fc0, w_fc1, _trace=False, _reps=1):
    global LAST_RESULTS
    x = np.ascontiguousarray(np.asarray(x, dtype=np.float32))
    w_fc0 = np.asarray(w_fc0, dtype=np.float32)
    w_fc1 = np.asarray(w_fc1, dtype=np.float32)

    nc = _build(_reps)
    mask0, wmx = _consts()
    w0T = np.ascontiguousarray(w_fc0.T)
    w1T = np.ascontiguousarray(w_fc1.T)

    in_maps = []
    for cid in range(8):
        bb, hb = cid // 2, cid % 2
        in_maps.append({
            "xin": np.ascontiguousarray(x[bb].reshape(C, T, HW)),
            "w0t": np.ascontiguousarray(w0T[:, 80 * hb:80 * hb + 80]),
            "w1t": np.ascontiguousarray(w1T[:, 80 * hb:80 * hb + 80]),
            "mask0": mask0,
            "wmx": wmx,
        })

    try:
        res = run_bass_kernel_spmd(nc, in_maps, core_ids=list(range(8)), trace=_trace)
    except ModuleNotFoundError:
        res = run_bass_kernel_spmd(nc, in_maps, core_ids=list(range(8)), trace=False)
    LAST_RESULTS = res

    out = np.zeros((B, 98, TP, H, W), np.float32)
    for cid in range(8):
        bb, hb = cid // 2, cid % 2
        t0 = hb * SLABS
        r = res.results[cid]
        o0 = r["out0"].reshape(SLABS, HW, K49).reshape(SLABS, H, W, K49)
        a0 = o0.transpose(0, 3, 1, 2)                      # [slab, k, y, x]
        out[bb, :49, t0:t0 + SLABS] = a0.transpose(1, 0, 2, 3)
        o1 = r["out1"].reshape(SLABS, 2, 2, 2, 8, 16, K49)  # [s, py, px, half, yyl, xx, k]
        a1 = o1.transpose(0, 6, 3, 4, 1, 5, 2).reshape(SLABS, K49, H, W)
        out[bb, 49:, t0:t0 + SLABS] = a1.transpose(1, 0, 2, 3)
    return out
